# revision 3
# baseline (speedup 1.0000x reference)
"""Trainium2 Bass kernel for nn_MultiHeadAttention_65352222376626 (v6: fp16 pass1 + fp8e5 DoubleRow residuals).

Algebra (host-fused): M8 = 8*Wq@Wk^T, A = Wv@Wp_h, exp-factorized bq bias
(G = 8*x@(Wk bq), EG=exp(G) folded into va), bv/bp folded into a host-side
output bias, softmax denominator appended as column 768 of va.

Numerics (numpy-validated, score rms err 0.0071 vs 0.0325 for the v4
fp32r+bf16 baseline):
  - tmat = x@M8: pass1 fp16 (m16 = fp16(M8), x16 = fp16(x); products exact)
    + pass2 fp8e5 DoubleRow, stationary [e5(M8-m16); e5(m16)] x moving
    [e5(x); e5(x-x16)] - BOTH residual terms in one half-rate pass.
  - t16 = fp16(tmat psum) is itself the scores pass1 stationary; residuals
    t8 = [e5(ps-t16); e5(t16)] feed the scores DoubleRow pass2 against the
    same fp8 x tile. DoubleRow = 0.5 cyc/row, so each residual pass costs
    half a regular pass.
  - va = x16 @ fp16(A) (more accurate than the old bf16 path); p in bf16;
    va_sb in bf16 (EG spans e^+-30, needs bf16 exponent range).
"""

import numpy as np
import ml_dtypes

B, S, D, H = 8, 1024, 768, 12
P = 128
SD = S // P   # 8 tiles along the sequence axis
ED = D // P   # 6 tiles along the feature axis

_CACHE = {}


def _build_nc():
    import concourse.tile as tile
    from concourse import bacc, mybir
    from concourse.masks import make_identity

    f32 = mybir.dt.float32
    fp16 = mybir.dt.float16
    bf16 = mybir.dt.bfloat16
    f8e5 = mybir.dt.float8e5
    AF = mybir.ActivationFunctionType
    DR = mybir.MatmulPerfMode.DoubleRow

    nc = bacc.Bacc()

    # ---- DRAM I/O (per core) ----
    xT_16_d = nc.dram_tensor("xT_16", [D, S], fp16, kind="ExternalInput")
    xT_8_d = nc.dram_tensor("xT_8", [D, 2, S], f8e5, kind="ExternalInput")
    m_16_d = nc.dram_tensor("m_16", [H, D, D], fp16, kind="ExternalInput")
    m_8_d = nc.dram_tensor("m_8", [H, D, 2, D], f8e5, kind="ExternalInput")
    a_d = nc.dram_tensor("a_w", [H, D, D], fp16, kind="ExternalInput")
    cT_d = nc.dram_tensor("cT_16", [D, H], fp16, kind="ExternalInput")
    out_d = nc.dram_tensor("out", [S, D], f32, kind="ExternalOutput")

    xT_16_t = xT_16_d.rearrange("(o p) s -> p o s", p=P)      # [128, ED, S]
    xT_8_t = xT_8_d.rearrange("(o p) two s -> p o two s", p=P)
    m_16_t = m_16_d.rearrange("h (o p) e -> h p o e", p=P)    # [H, 128, ED, D]
    m_8_t = m_8_d.rearrange("h (o p) two e -> h p o two e", p=P)
    a_t = a_d.rearrange("h (o p) e -> h p o e", p=P)
    cT_t = cT_d.rearrange("(o p) h -> p o h", p=P)            # [128, ED, H]
    out_t = out_d.rearrange("(o p) d -> p o d", p=P)          # [128, SD, D]

    with tile.TileContext(nc) as tc:
        with (
            tc.tile_pool(name="persist", bufs=1) as persist,
            tc.tile_pool(name="whead", bufs=2) as whead,
            tc.tile_pool(name="qk", bufs=1) as qkpool,
            tc.tile_pool(name="work", bufs=2) as work,
            tc.tile_pool(name="small", bufs=4) as small,
            tc.tile_pool(name="scps", bufs=2, space="PSUM") as scps,
            tc.tile_pool(name="mmps", bufs=2, space="PSUM") as mmps,
            tc.tile_pool(name="prps", bufs=1, space="PSUM") as prps,
        ):
            # ---- persistent tiles ----
            cf = persist.tile([P, ED, H], fp16)
            nc.sync.dma_start(cf[:], cT_t)
            x16 = persist.tile([P, ED, S], fp16)
            nc.sync.dma_start(x16[:, :, 0:512], xT_16_t[:, :, 0:512])
            nc.sync.dma_start(x16[:, :, 512:1024], xT_16_t[:, :, 512:1024])
            x8 = persist.tile([P, ED, 2, S], f8e5)
            nc.sync.dma_start(x8[:], xT_8_t)

            ident = persist.tile([P, P], bf16)
            make_identity(nc, ident)
            ident12 = persist.tile([H, H], f32)
            make_identity(nc, ident12)

            acc = persist.tile([P, SD, D], f32)      # final accumulator
            egt = persist.tile([P, SD, H], f32)      # exp(G) transposed
            t16a = qkpool.tile([P, ED, S], fp16)     # tmat hi (fp16 grid)
            t8 = qkpool.tile([P, ED, 2, S], f8e5)    # [tlo; t16] fp8 residuals
            va_sb = qkpool.tile([P, SD, D + 1], bf16)

            # ---- G = 8*x@(Wk bq) all heads (fp16); EG = exp(G) ----
            eg_sb = work.tile([H, S], f32, tag="scratch", bufs=1)
            g0 = mmps.tile([H, 512], f32, tag="mm")
            g1 = mmps.tile([H, 512], f32, tag="mm")
            for dt in range(ED):
                sh = cf[:, dt, :]
                nc.tensor.matmul(g0[:], sh, x16[:, dt, 0:512],
                                 start=(dt == 0), stop=(dt == ED - 1))
                nc.tensor.matmul(g1[:], sh, x16[:, dt, 512:1024],
                                 start=(dt == 0), stop=(dt == ED - 1))
            nc.scalar.activation(eg_sb[:, 0:512], g0[:], AF.Exp)
            nc.scalar.activation(eg_sb[:, 512:1024], g1[:], AF.Exp)
            for tt in range(SD):
                t_sl = slice(tt * P, (tt + 1) * P)
                egp = mmps.tile([P, H], f32, tag="mm")
                nc.tensor.transpose(egp[:], eg_sb[:, t_sl], ident12[:])
                nc.scalar.copy(egt[:, tt, :], egp[:])

            for h in range(H):
                # ---- per-head weight loads (double-buffered) ----
                m16 = whead.tile([P, ED, D], fp16, tag="m16")
                nc.sync.dma_start(m16[:], m_16_t[h])
                m8 = whead.tile([P, ED, 2, D], f8e5, tag="m8")
                nc.sync.dma_start(m8[:], m_8_t[h])
                aw = whead.tile([P, ED, D], fp16, tag="aw")
                nc.sync.dma_start(aw[:], a_t[h])

                # ---- tmat = x @ M8_h: fp16 + fp8e5-DR residual ----
                for et in range(ED):
                    e_sl = slice(et * P, (et + 1) * P)
                    ps = scps.tile([P, S], f32, tag="sc")
                    for dt in range(ED):
                        sf = m16[:, dt, e_sl]
                        nc.tensor.matmul(ps[:, 0:512], sf, x16[:, dt, 0:512],
                                         start=(dt == 0), stop=False)
                        nc.tensor.matmul(ps[:, 512:1024], sf,
                                         x16[:, dt, 512:1024],
                                         start=(dt == 0), stop=False)
                        s8 = m8[:, dt, :, e_sl]
                        nc.tensor.matmul(ps[:, 0:512], s8,
                                         x8[:, dt, :, 0:512],
                                         start=False, stop=(dt == ED - 1),
                                         perf_mode=DR)
                        nc.tensor.matmul(ps[:, 512:1024], s8,
                                         x8[:, dt, :, 512:1024],
                                         start=False, stop=(dt == ED - 1),
                                         perf_mode=DR)
                    # split on the fp16 grid: t16a = fp16(ps) is the scores
                    # pass1 stationary; residuals to fp8e5.
                    nc.scalar.activation(t16a[:, et, :], ps[:], AF.Copy)
                    nc.vector.tensor_sub(t8[:, et, 0, :], ps[:],
                                         t16a[:, et, :])
                    nc.scalar.activation(t8[:, et, 1, :], t16a[:, et, :],
                                         AF.Copy)

                # ---- va = (x @ A_h) * EG, with EG appended as col 768 ----
                for tt in range(SD):
                    t_sl = slice(tt * P, (tt + 1) * P)
                    va0 = mmps.tile([P, 512], f32, tag="mm")
                    va1 = mmps.tile([P, 256], f32, tag="mm")
                    for dt in range(ED):
                        sx = x16[:, dt, t_sl]
                        nc.tensor.matmul(va0[:], sx, aw[:, dt, 0:512],
                                         start=(dt == 0), stop=(dt == ED - 1))
                        nc.tensor.matmul(va1[:], sx, aw[:, dt, 512:768],
                                         start=(dt == 0), stop=(dt == ED - 1))
                    sc = egt[:, tt, h:h + 1]
                    nc.scalar.mul(va_sb[:, tt, 0:512], va0[:], sc)
                    nc.scalar.mul(va_sb[:, tt, 512:768], va1[:], sc)
                    nc.vector.tensor_copy(va_sb[:, tt, 768:769], sc)

                # ---- scores / softmax / transpose / out-proj, pipelined ----
                def tail(st, ptile):
                    s_sl = slice(st * P, (st + 1) * P)
                    pTs = work.tile([P, SD, P], bf16, tag="pT")
                    for tt in range(SD):
                        t_sl = slice(tt * P, (tt + 1) * P)
                        tpp = mmps.tile([P, 512], bf16, tag="mm")
                        nc.tensor.transpose(tpp[:, 0:P], ptile[:, t_sl],
                                            ident[:])
                        nc.vector.tensor_copy(pTs[:, tt, :], tpp[:, 0:P])
                    pr = prps.tile([P, D + 1], f32, tag="pr")
                    for tt in range(SD):
                        sp = pTs[:, tt, :]
                        nc.tensor.matmul(pr[:, 0:512], sp,
                                         va_sb[:, tt, 0:512],
                                         start=(tt == 0), stop=False)
                        nc.tensor.matmul(pr[:, 512:769], sp,
                                         va_sb[:, tt, 512:769],
                                         start=(tt == 0), stop=(tt == SD - 1))
                    rc = small.tile([P, 1], f32, tag="rc")
                    nc.vector.reciprocal(rc[:], pr[:, D:D + 1])
                    if h == 0:
                        nc.scalar.mul(acc[:, st, :], pr[:, 0:D], rc[:])
                    else:
                        tmp = work.tile([P, D], f32, tag="tmp", bufs=1)
                        nc.scalar.mul(tmp[:], pr[:, 0:D], rc[:])
                        nc.vector.tensor_add(acc[:, st, :], acc[:, st, :],
                                             tmp[:])
                    if h == H - 1:
                        nc.sync.dma_start(out_t[:, st, :], acc[:, st, :])

                prev = None
                for st in range(SD):
                    s_sl = slice(st * P, (st + 1) * P)
                    sc_ps = scps.tile([P, S], f32, tag="sc")
                    for et in range(ED):
                        sf = t16a[:, et, s_sl]
                        nc.tensor.matmul(sc_ps[:, 0:512], sf,
                                         x16[:, et, 0:512],
                                         start=(et == 0), stop=False)
                        nc.tensor.matmul(sc_ps[:, 512:1024], sf,
                                         x16[:, et, 512:1024],
                                         start=(et == 0), stop=False)
                        s8 = t8[:, et, :, s_sl]
                        nc.tensor.matmul(sc_ps[:, 0:512], s8,
                                         x8[:, et, :, 0:512],
                                         start=False, stop=(et == ED - 1),
                                         perf_mode=DR)
                        nc.tensor.matmul(sc_ps[:, 512:1024], s8,
                                         x8[:, et, :, 512:1024],
                                         start=False, stop=(et == ED - 1),
                                         perf_mode=DR)
                    negmax = small.tile([P, 1], f32, tag="negmax")
                    nc.vector.tensor_reduce(
                        negmax[:], sc_ps[:], axis=mybir.AxisListType.X,
                        op=mybir.AluOpType.max, negate=True)
                    ptile = work.tile([P, S], bf16, tag="p")
                    nc.scalar.activation(ptile[:], sc_ps[:], AF.Exp,
                                         bias=negmax[:])
                    if prev is not None:
                        tail(*prev)
                    prev = (st, ptile)
                tail(*prev)

    nc.compile()
    return nc


def _get_nc():
    if "nc" not in _CACHE:
        _CACHE["nc"] = _build_nc()
    return _CACHE["nc"]


def _prepare(x, Wq, bq, Wk, bk, Wv, bv, Wp, bp):
    x = np.asarray(x, dtype=np.float32)
    Wq = np.asarray(Wq, dtype=np.float32)
    Wk = np.asarray(Wk, dtype=np.float32)
    Wv = np.asarray(Wv, dtype=np.float32)
    Wp = np.asarray(Wp, dtype=np.float32)
    bq = np.asarray(bq, dtype=np.float32)
    bv = np.asarray(bv, dtype=np.float32)
    bp = np.asarray(bp, dtype=np.float32)

    wp3 = Wp.reshape(H, D, D)
    M8 = 8.0 * np.matmul(Wq, np.transpose(Wk, (0, 2, 1)))
    A = np.matmul(Wv, wp3)
    c8 = 8.0 * np.einsum('hde,he->hd', Wk, bq)
    bp_eff = (bp.astype(np.float64)
              + np.einsum('hd,hde->e', bv.astype(np.float64),
                          wp3.astype(np.float64))).astype(np.float32)

    m16 = M8.astype(np.float16)
    m_8 = np.empty((H, D, 2, D), dtype=ml_dtypes.float8_e5m2)
    m_8[:, :, 0, :] = (M8 - m16.astype(np.float32)).astype(
        ml_dtypes.float8_e5m2)
    m_8[:, :, 1, :] = m16.astype(ml_dtypes.float8_e5m2)
    a_16 = A.astype(np.float16)
    cT_16 = np.ascontiguousarray(c8.T).astype(np.float16)  # [D, H]

    shared = {
        "m_16": m16, "m_8": m_8, "a_w": a_16, "cT_16": cT_16,
    }
    in_maps = []
    for b in range(B):
        xT = np.ascontiguousarray(x[b].T)
        xT16 = xT.astype(np.float16)
        x_8 = np.empty((D, 2, S), dtype=ml_dtypes.float8_e5m2)
        x_8[:, 0, :] = xT.astype(ml_dtypes.float8_e5m2)
        x_8[:, 1, :] = (xT - xT16.astype(np.float32)).astype(
            ml_dtypes.float8_e5m2)
        m = {"xT_16": xT16, "xT_8": x_8, **shared}
        in_maps.append(m)
    return in_maps, bp_eff


def kernel(x, Wq, bq, Wk, bk, Wv, bv, Wp, bp):
    from concourse.bass_utils import run_bass_kernel_spmd

    in_maps, bp_eff = _prepare(x, Wq, bq, Wk, bk, Wv, bv, Wp, bp)
    nc = _get_nc()
    res = run_bass_kernel_spmd(nc, in_maps, list(range(B)))
    out = np.stack([res.results[b]["out"] for b in range(B)], axis=0)
    out = out + bp_eff[None, None, :]
    return out.astype(np.float32)


# revision 4
# speedup vs baseline: 1.1552x; 1.1552x over previous
"""Trainium2 Bass kernel for nn_MultiHeadAttention_65352222376626 (v7: fp32r pass1 + paired fp8e5 DoubleRow residual).

Algebra (host-fused): M8 = 8*Wq@Wk^T, A = Wv@Wp_h, exp-factorized bq bias
(G = 8*x@(Wk bq), EG=exp(G) folded into va), bv/bp folded into a host-side
output bias, softmax denominator appended as column 768 of va.

Numerics:
  - tmat = x@M8: pass1 fp32r (q12(M8) x q12(x)) + pass2 fp8e5 DoubleRow:
    each DR instruction contracts TWO adjacent 128-chunks of the single
    residual term e5(M8-q12(M8)) x e5(x), so pass2 costs half of a bf16
    pass (HW: DR streams 2 k-chunks per output row at ~1 cyc/row).
  - t16 = fp16(tmat psum) lies on the fp32r q12 grid, so the scores pass1
    stationary upcast(t16) passes through fp32r unrounded; the residual
    tlo = e5(ps - t16) feeds the paired-DR scores pass2 against e5(x).
  - score rms err ~0.033 (numpy) - same as the v4 baseline that measured
    0.0118 output rel err; va in fp16 (was bf16) buys a little back.
  - G in fp32r; p/va_sb bf16 (EG spans e^+-30, needs bf16 exponent range).
"""

import numpy as np
import ml_dtypes

B, S, D, H = 8, 1024, 768, 12
P = 128
SD = S // P   # 8 tiles along the sequence axis
ED = D // P   # 6 tiles along the feature axis

_CACHE = {}


def _build_nc():
    import concourse.tile as tile
    from concourse import bacc, mybir
    from concourse.masks import make_identity

    f32 = mybir.dt.float32
    f32r = mybir.dt.float32r
    fp16 = mybir.dt.float16
    bf16 = mybir.dt.bfloat16
    f8e5 = mybir.dt.float8e5
    AF = mybir.ActivationFunctionType
    DR = mybir.MatmulPerfMode.DoubleRow

    nc = bacc.Bacc()

    # ---- DRAM I/O (per core) ----
    xT_f_d = nc.dram_tensor("xT_f", [D, S], f32r, kind="ExternalInput")
    xT_16_d = nc.dram_tensor("xT_16", [D, S], fp16, kind="ExternalInput")
    xT_8_d = nc.dram_tensor("xT_8", [D, S], f8e5, kind="ExternalInput")
    m_f_d = nc.dram_tensor("m_f", [H, D, D], f32r, kind="ExternalInput")
    m_8_d = nc.dram_tensor("m_8", [H, D, D], f8e5, kind="ExternalInput")
    a_d = nc.dram_tensor("a_w", [H, D, D], fp16, kind="ExternalInput")
    cT_f_d = nc.dram_tensor("cT_f", [D, H], f32r, kind="ExternalInput")
    out_d = nc.dram_tensor("out", [S, D], f32, kind="ExternalOutput")

    xT_f_t = xT_f_d.rearrange("(o p) s -> p o s", p=P)        # [128, ED, S]
    xT_16_t = xT_16_d.rearrange("(o p) s -> p o s", p=P)
    xT_8_t = xT_8_d.rearrange("(o p) s -> p o s", p=P)
    m_f_t = m_f_d.rearrange("h (o p) e -> h p o e", p=P)      # [H, 128, ED, D]
    m_8_t = m_8_d.rearrange("h (o p) e -> h p o e", p=P)
    a_t = a_d.rearrange("h (o p) e -> h p o e", p=P)
    cT_f_t = cT_f_d.rearrange("(o p) h -> p o h", p=P)        # [128, ED, H]
    out_t = out_d.rearrange("(o p) d -> p o d", p=P)          # [128, SD, D]

    with tile.TileContext(nc) as tc:
        with (
            tc.tile_pool(name="persist", bufs=1) as persist,
            tc.tile_pool(name="whead", bufs=2) as whead,
            tc.tile_pool(name="qk", bufs=1) as qkpool,
            tc.tile_pool(name="work", bufs=2) as work,
            tc.tile_pool(name="small", bufs=4) as small,
            tc.tile_pool(name="scps", bufs=2, space="PSUM") as scps,
            tc.tile_pool(name="mmps", bufs=2, space="PSUM") as mmps,
            tc.tile_pool(name="prps", bufs=1, space="PSUM") as prps,
        ):
            # ---- persistent tiles; DMA order tuned so head-0 compute can
            # start as early as possible ----
            cf = persist.tile([P, ED, H], f32r)
            nc.sync.dma_start(cf[:], cT_f_t)
            xf = persist.tile([P, ED, S], f32r)
            nc.sync.dma_start(xf[:, :, 0:512], xT_f_t[:, :, 0:512])
            nc.sync.dma_start(xf[:, :, 512:1024], xT_f_t[:, :, 512:1024])

            # head-0 weights issued before the fp8/fp16 x copies
            mf0 = whead.tile([P, ED, D], f32r, tag="mf")
            nc.sync.dma_start(mf0[:], m_f_t[0])
            x8 = persist.tile([P, ED, S], f8e5)
            nc.sync.dma_start(x8[:], xT_8_t)
            m80 = whead.tile([P, ED, D], f8e5, tag="m8")
            nc.sync.dma_start(m80[:], m_8_t[0])
            x16 = persist.tile([P, ED, S], fp16)
            nc.sync.dma_start(x16[:], xT_16_t)
            aw0 = whead.tile([P, ED, D], fp16, tag="aw")
            nc.sync.dma_start(aw0[:], a_t[0])

            ident = persist.tile([P, P], bf16)
            make_identity(nc, ident)
            ident12 = persist.tile([H, H], f32)
            make_identity(nc, ident12)

            acc = persist.tile([P, SD, D], f32)      # final accumulator
            egt = persist.tile([P, SD, H], f32)      # exp(G) transposed
            tf32 = qkpool.tile([P, ED, S], f32r)     # tmat hi (fp16 grid)
            t8 = qkpool.tile([P, ED, S], f8e5)       # tmat residual fp8
            va_sb = qkpool.tile([P, SD, D + 1], bf16)

            # ---- G = 8*x@(Wk bq) all heads (1-pass fp32r); EG = exp(G) ----
            eg_sb = work.tile([H, S], f32, tag="scratch", bufs=1)
            g0 = mmps.tile([H, 512], f32, tag="mm")
            g1 = mmps.tile([H, 512], f32, tag="mm")
            for dt in range(ED):
                sh = cf[:, dt, :]
                nc.tensor.matmul(g0[:], sh, xf[:, dt, 0:512],
                                 start=(dt == 0), stop=(dt == ED - 1))
                nc.tensor.matmul(g1[:], sh, xf[:, dt, 512:1024],
                                 start=(dt == 0), stop=(dt == ED - 1))
            nc.scalar.activation(eg_sb[:, 0:512], g0[:], AF.Exp)
            nc.scalar.activation(eg_sb[:, 512:1024], g1[:], AF.Exp)

            def eg_transposes():
                for tt in range(SD):
                    t_sl = slice(tt * P, (tt + 1) * P)
                    egp = mmps.tile([P, H], f32, tag="mm")
                    nc.tensor.transpose(egp[:], eg_sb[:, t_sl], ident12[:])
                    nc.scalar.copy(egt[:, tt, :], egp[:])

            cur = (mf0, m80, aw0)
            for h in range(H):
                mf, m8, aw = cur
                if h + 1 < H:
                    mfn = whead.tile([P, ED, D], f32r, tag="mf")
                    nc.sync.dma_start(mfn[:], m_f_t[h + 1])
                    m8n = whead.tile([P, ED, D], f8e5, tag="m8")
                    nc.sync.dma_start(m8n[:], m_8_t[h + 1])
                    awn = whead.tile([P, ED, D], fp16, tag="aw")
                    nc.sync.dma_start(awn[:], a_t[h + 1])
                    cur = (mfn, m8n, awn)

                # ---- tmat = x @ M8_h: fp32r pass + paired-DR fp8 pass ----
                for et in range(ED):
                    e_sl = slice(et * P, (et + 1) * P)
                    ps = scps.tile([P, S], f32, tag="sc")
                    for dt in range(ED):
                        sf = mf[:, dt, e_sl]
                        nc.tensor.matmul(ps[:, 0:512], sf, xf[:, dt, 0:512],
                                         start=(dt == 0), stop=False)
                        nc.tensor.matmul(ps[:, 512:1024], sf,
                                         xf[:, dt, 512:1024],
                                         start=(dt == 0), stop=False)
                    for dp in range(ED // 2):
                        d_sl = slice(2 * dp, 2 * dp + 2)
                        s8 = m8[:, d_sl, e_sl]
                        nc.tensor.matmul(ps[:, 0:512], s8,
                                         x8[:, d_sl, 0:512],
                                         start=False, stop=(dp == 2),
                                         perf_mode=DR)
                        nc.tensor.matmul(ps[:, 512:1024], s8,
                                         x8[:, d_sl, 512:1024],
                                         start=False, stop=(dp == 2),
                                         perf_mode=DR)
                    # split: t16 on the fp16 grid (exact under fp32r q12);
                    # tlo = psum - t16 to fp8e5; tf32 = upcast(t16).
                    t16 = work.tile([P, S], fp16, tag="t16")
                    nc.scalar.activation(t16[:], ps[:], AF.Copy)
                    nc.scalar.activation(tf32[:, et, :], t16[:], AF.Copy)
                    nc.vector.tensor_sub(t8[:, et, :], ps[:], tf32[:, et, :])

                if h == 0:
                    eg_transposes()

                # ---- va = (x @ A_h) * EG, with EG appended as col 768 ----
                for tt in range(SD):
                    t_sl = slice(tt * P, (tt + 1) * P)
                    va0 = mmps.tile([P, 512], f32, tag="mm")
                    va1 = mmps.tile([P, 256], f32, tag="mm")
                    for dt in range(ED):
                        sx = x16[:, dt, t_sl]
                        nc.tensor.matmul(va0[:], sx, aw[:, dt, 0:512],
                                         start=(dt == 0), stop=(dt == ED - 1))
                        nc.tensor.matmul(va1[:], sx, aw[:, dt, 512:768],
                                         start=(dt == 0), stop=(dt == ED - 1))
                    sc = egt[:, tt, h:h + 1]
                    nc.scalar.mul(va_sb[:, tt, 0:512], va0[:], sc)
                    nc.scalar.mul(va_sb[:, tt, 512:768], va1[:], sc)
                    nc.vector.tensor_copy(va_sb[:, tt, 768:769], sc)

                # ---- scores / softmax / transpose / out-proj, pipelined ----
                def tail(st, ptile):
                    s_sl = slice(st * P, (st + 1) * P)
                    pTs = work.tile([P, SD, P], bf16, tag="pT")
                    for tt in range(SD):
                        t_sl = slice(tt * P, (tt + 1) * P)
                        tpp = mmps.tile([P, 512], bf16, tag="mm")
                        nc.tensor.transpose(tpp[:, 0:P], ptile[:, t_sl],
                                            ident[:])
                        nc.vector.tensor_copy(pTs[:, tt, :], tpp[:, 0:P])
                    pr = prps.tile([P, D + 1], f32, tag="pr")
                    for tt in range(SD):
                        sp = pTs[:, tt, :]
                        nc.tensor.matmul(pr[:, 0:512], sp,
                                         va_sb[:, tt, 0:512],
                                         start=(tt == 0), stop=False)
                        nc.tensor.matmul(pr[:, 512:769], sp,
                                         va_sb[:, tt, 512:769],
                                         start=(tt == 0), stop=(tt == SD - 1))
                    rc = small.tile([P, 1], f32, tag="rc")
                    nc.vector.reciprocal(rc[:], pr[:, D:D + 1])
                    if h == 0:
                        nc.scalar.mul(acc[:, st, :], pr[:, 0:D], rc[:])
                    else:
                        tmp = work.tile([P, D], f32, tag="tmp", bufs=1)
                        nc.scalar.mul(tmp[:], pr[:, 0:D], rc[:])
                        nc.vector.tensor_add(acc[:, st, :], acc[:, st, :],
                                             tmp[:])
                    if h == H - 1:
                        nc.sync.dma_start(out_t[:, st, :], acc[:, st, :])

                prev = None
                for st in range(SD):
                    s_sl = slice(st * P, (st + 1) * P)
                    sc_ps = scps.tile([P, S], f32, tag="sc")
                    for et in range(ED):
                        sf = tf32[:, et, s_sl]
                        nc.tensor.matmul(sc_ps[:, 0:512], sf,
                                         xf[:, et, 0:512],
                                         start=(et == 0), stop=False)
                        nc.tensor.matmul(sc_ps[:, 512:1024], sf,
                                         xf[:, et, 512:1024],
                                         start=(et == 0), stop=False)
                    for ep in range(ED // 2):
                        e_2 = slice(2 * ep, 2 * ep + 2)
                        s8 = t8[:, e_2, s_sl]
                        nc.tensor.matmul(sc_ps[:, 0:512], s8,
                                         x8[:, e_2, 0:512],
                                         start=False, stop=(ep == 2),
                                         perf_mode=DR)
                        nc.tensor.matmul(sc_ps[:, 512:1024], s8,
                                         x8[:, e_2, 512:1024],
                                         start=False, stop=(ep == 2),
                                         perf_mode=DR)
                    negmax = small.tile([P, 1], f32, tag="negmax")
                    nc.vector.tensor_reduce(
                        negmax[:], sc_ps[:], axis=mybir.AxisListType.X,
                        op=mybir.AluOpType.max, negate=True)
                    ptile = work.tile([P, S], bf16, tag="p")
                    nc.scalar.activation(ptile[:], sc_ps[:], AF.Exp,
                                         bias=negmax[:])
                    if prev is not None:
                        tail(*prev)
                    prev = (st, ptile)
                tail(*prev)

    nc.compile()
    return nc


def _get_nc():
    if "nc" not in _CACHE:
        _CACHE["nc"] = _build_nc()
    return _CACHE["nc"]


def _q12(a):
    """Round fp32 mantissa to 12 bits RNE (matches TRN2 fp32r operand quant)."""
    a32 = np.asarray(a, np.float32)
    c = np.float32(2 ** 12 + 1)
    s = (a32 * c).astype(np.float32)
    return (s - (s - a32).astype(np.float32)).astype(np.float32)


def _prepare(x, Wq, bq, Wk, bk, Wv, bv, Wp, bp):
    x = np.asarray(x, dtype=np.float32)
    Wq = np.asarray(Wq, dtype=np.float32)
    Wk = np.asarray(Wk, dtype=np.float32)
    Wv = np.asarray(Wv, dtype=np.float32)
    Wp = np.asarray(Wp, dtype=np.float32)
    bq = np.asarray(bq, dtype=np.float32)
    bv = np.asarray(bv, dtype=np.float32)
    bp = np.asarray(bp, dtype=np.float32)

    wp3 = Wp.reshape(H, D, D)
    M8 = 8.0 * np.matmul(Wq, np.transpose(Wk, (0, 2, 1)))
    A = np.matmul(Wv, wp3)
    c8 = 8.0 * np.einsum('hde,he->hd', Wk, bq)
    bp_eff = (bp.astype(np.float64)
              + np.einsum('hd,hde->e', bv.astype(np.float64),
                          wp3.astype(np.float64))).astype(np.float32)

    m_8 = (M8 - _q12(M8)).astype(ml_dtypes.float8_e5m2)
    a_16 = A.astype(np.float16)
    cT_f = np.ascontiguousarray(c8.T)  # [D, H] fp32

    shared = {
        "m_f": M8, "m_8": m_8, "a_w": a_16, "cT_f": cT_f,
    }
    in_maps = []
    for b in range(B):
        xT = np.ascontiguousarray(x[b].T)
        m = {"xT_f": xT, "xT_16": xT.astype(np.float16),
             "xT_8": xT.astype(ml_dtypes.float8_e5m2), **shared}
        in_maps.append(m)
    return in_maps, bp_eff


def kernel(x, Wq, bq, Wk, bk, Wv, bv, Wp, bp):
    from concourse.bass_utils import run_bass_kernel_spmd

    in_maps, bp_eff = _prepare(x, Wq, bq, Wk, bk, Wv, bv, Wp, bp)
    nc = _get_nc()
    res = run_bass_kernel_spmd(nc, in_maps, list(range(B)))
    out = np.stack([res.results[b]["out"] for b in range(B)], axis=0)
    out = out + bp_eff[None, None, :]
    return out.astype(np.float32)


# revision 6
# speedup vs baseline: 1.1567x; 1.0013x over previous
"""Trainium2 Bass kernel for nn_MultiHeadAttention_65352222376626 (v7: fp32r pass1 + paired fp8e5 DoubleRow residual).

Algebra (host-fused): M8 = 8*Wq@Wk^T, A = Wv@Wp_h, exp-factorized bq bias
(G = 8*x@(Wk bq), EG=exp(G) folded into va), bv/bp folded into a host-side
output bias, softmax denominator appended as column 768 of va.

Numerics:
  - tmat = x@M8: pass1 fp32r (q12(M8) x q12(x)) + pass2 fp8e5 DoubleRow:
    each DR instruction contracts TWO adjacent 128-chunks of the single
    residual term e5(M8-q12(M8)) x e5(x), so pass2 costs half of a bf16
    pass (HW: DR streams 2 k-chunks per output row at ~1 cyc/row).
  - t16 = fp16(tmat psum) lies on the fp32r q12 grid, so the scores pass1
    stationary upcast(t16) passes through fp32r unrounded; the residual
    tlo = e5(ps - t16) feeds the paired-DR scores pass2 against e5(x).
  - score rms err ~0.033 (numpy) - same as the v4 baseline that measured
    0.0118 output rel err; va in fp16 (was bf16) buys a little back.
  - G in fp32r; p/va_sb bf16 (EG spans e^+-30, needs bf16 exponent range).
"""

import numpy as np
import ml_dtypes

B, S, D, H = 8, 1024, 768, 12
P = 128
SD = S // P   # 8 tiles along the sequence axis
ED = D // P   # 6 tiles along the feature axis

_CACHE = {}


def _build_nc():
    import concourse.tile as tile
    from concourse import bacc, mybir
    from concourse.masks import make_identity

    f32 = mybir.dt.float32
    f32r = mybir.dt.float32r
    fp16 = mybir.dt.float16
    bf16 = mybir.dt.bfloat16
    f8e5 = mybir.dt.float8e5
    AF = mybir.ActivationFunctionType
    DR = mybir.MatmulPerfMode.DoubleRow

    nc = bacc.Bacc()

    # ---- DRAM I/O (per core) ----
    xT_f_d = nc.dram_tensor("xT_f", [D, S], f32r, kind="ExternalInput")
    xT_16_d = nc.dram_tensor("xT_16", [D, S], fp16, kind="ExternalInput")
    xT_8_d = nc.dram_tensor("xT_8", [D, S], f8e5, kind="ExternalInput")
    m_f_d = nc.dram_tensor("m_f", [H, D, D], f32r, kind="ExternalInput")
    m_8_d = nc.dram_tensor("m_8", [H, D, D], f8e5, kind="ExternalInput")
    a_d = nc.dram_tensor("a_w", [H, D, D], fp16, kind="ExternalInput")
    cT_f_d = nc.dram_tensor("cT_f", [D, H], f32r, kind="ExternalInput")
    out_d = nc.dram_tensor("out", [S, D], f32, kind="ExternalOutput")

    xT_f_t = xT_f_d.rearrange("(o p) s -> p o s", p=P)        # [128, ED, S]
    xT_16_t = xT_16_d.rearrange("(o p) s -> p o s", p=P)
    xT_8_t = xT_8_d.rearrange("(o p) s -> p o s", p=P)
    m_f_t = m_f_d.rearrange("h (o p) e -> h p o e", p=P)      # [H, 128, ED, D]
    m_8_t = m_8_d.rearrange("h (o p) e -> h p o e", p=P)
    a_t = a_d.rearrange("h (o p) e -> h p o e", p=P)
    cT_f_t = cT_f_d.rearrange("(o p) h -> p o h", p=P)        # [128, ED, H]
    out_t = out_d.rearrange("(o p) d -> p o d", p=P)          # [128, SD, D]

    with tile.TileContext(nc) as tc:
        with (
            tc.tile_pool(name="persist", bufs=1) as persist,
            tc.tile_pool(name="whead", bufs=2) as whead,
            tc.tile_pool(name="qk", bufs=1) as qkpool,
            tc.tile_pool(name="work", bufs=2) as work,
            tc.tile_pool(name="small", bufs=4) as small,
            tc.tile_pool(name="scps", bufs=2, space="PSUM") as scps,
            tc.tile_pool(name="mmps", bufs=2, space="PSUM") as mmps,
            tc.tile_pool(name="prps", bufs=1, space="PSUM") as prps,
        ):
            # ---- persistent tiles; DMA order tuned so head-0 compute can
            # start as early as possible ----
            cf = persist.tile([P, ED, H], f32r)
            nc.sync.dma_start(cf[:], cT_f_t)
            xf = persist.tile([P, ED, S], f32r)
            nc.sync.dma_start(xf[:, :, 0:512], xT_f_t[:, :, 0:512])
            nc.sync.dma_start(xf[:, :, 512:1024], xT_f_t[:, :, 512:1024])

            # head-0 weights issued before the fp8/fp16 x copies; the first
            # et-slice of m_f[0] goes first so tmat-h0 can start early
            mf0 = whead.tile([P, ED, D], f32r, tag="mf")
            nc.sync.dma_start(mf0[:, :, 0:P], m_f_t[0][:, :, 0:P])
            x8 = persist.tile([P, ED, S], f8e5)
            nc.sync.dma_start(x8[:], xT_8_t)
            m80 = whead.tile([P, ED, D], f8e5, tag="m8")
            nc.sync.dma_start(m80[:], m_8_t[0])
            nc.sync.dma_start(mf0[:, :, P:D], m_f_t[0][:, :, P:D])
            x16 = persist.tile([P, ED, S], fp16)
            nc.sync.dma_start(x16[:], xT_16_t)
            aw0 = whead.tile([P, ED, D], fp16, tag="aw")
            nc.sync.dma_start(aw0[:], a_t[0])

            ident = persist.tile([P, P], bf16)
            make_identity(nc, ident)
            ident12 = persist.tile([H, H], f32)
            make_identity(nc, ident12)

            acc = persist.tile([P, SD, D], f32)      # final accumulator
            egt = persist.tile([P, SD, H], f32)      # exp(G) transposed
            tf32 = qkpool.tile([P, ED, S], f32r)     # tmat hi (fp16 grid)
            t8 = qkpool.tile([P, ED, S], f8e5)       # tmat residual fp8
            va_sb = qkpool.tile([P, SD, D + 1], bf16)

            # ---- G = 8*x@(Wk bq) all heads (1-pass fp32r); EG = exp(G) ----
            eg_sb = work.tile([H, S], f32, tag="scratch", bufs=1)
            g0 = mmps.tile([H, 512], f32, tag="mm")
            g1 = mmps.tile([H, 512], f32, tag="mm")
            for dt in range(ED):
                nc.tensor.matmul(g0[:], cf[:, dt, :], xf[:, dt, 0:512],
                                 start=(dt == 0), stop=(dt == ED - 1))
            for dt in range(ED):
                nc.tensor.matmul(g1[:], cf[:, dt, :], xf[:, dt, 512:1024],
                                 start=(dt == 0), stop=(dt == ED - 1))
            nc.scalar.activation(eg_sb[:, 0:512], g0[:], AF.Exp)
            nc.scalar.activation(eg_sb[:, 512:1024], g1[:], AF.Exp)

            def eg_transposes():
                for tt in range(SD):
                    t_sl = slice(tt * P, (tt + 1) * P)
                    egp = mmps.tile([P, H], f32, tag="mm")
                    nc.tensor.transpose(egp[:], eg_sb[:, t_sl], ident12[:])
                    nc.scalar.copy(egt[:, tt, :], egp[:])

            cur = (mf0, m80, aw0)
            for h in range(H):
                mf, m8, aw = cur
                if h + 1 < H:
                    mfn = whead.tile([P, ED, D], f32r, tag="mf")
                    nc.sync.dma_start(mfn[:], m_f_t[h + 1])
                    m8n = whead.tile([P, ED, D], f8e5, tag="m8")
                    nc.sync.dma_start(m8n[:], m_8_t[h + 1])
                    awn = whead.tile([P, ED, D], fp16, tag="aw")
                    nc.sync.dma_start(awn[:], a_t[h + 1])
                    cur = (mfn, m8n, awn)

                # ---- tmat = x @ M8_h: fp32r pass + paired-DR fp8 pass ----
                for et in range(ED):
                    e_sl = slice(et * P, (et + 1) * P)
                    ps = scps.tile([P, S], f32, tag="sc")
                    for dt in range(ED):
                        sf = mf[:, dt, e_sl]
                        nc.tensor.matmul(ps[:, 0:512], sf, xf[:, dt, 0:512],
                                         start=(dt == 0), stop=False)
                        nc.tensor.matmul(ps[:, 512:1024], sf,
                                         xf[:, dt, 512:1024],
                                         start=(dt == 0), stop=False)
                    for dp in range(ED // 2):
                        d_sl = slice(2 * dp, 2 * dp + 2)
                        s8 = m8[:, d_sl, e_sl]
                        nc.tensor.matmul(ps[:, 0:512], s8,
                                         x8[:, d_sl, 0:512],
                                         start=False, stop=(dp == 2),
                                         perf_mode=DR)
                        nc.tensor.matmul(ps[:, 512:1024], s8,
                                         x8[:, d_sl, 512:1024],
                                         start=False, stop=(dp == 2),
                                         perf_mode=DR)
                    # split: t16 on the fp16 grid (exact under fp32r q12);
                    # tlo = psum - t16 to fp8e5; tf32 = upcast(t16).
                    t16 = work.tile([P, S], fp16, tag="t16")
                    nc.scalar.activation(t16[:], ps[:], AF.Copy)
                    nc.scalar.activation(tf32[:, et, :], t16[:], AF.Copy)
                    nc.vector.tensor_sub(t8[:, et, :], ps[:], tf32[:, et, :])

                if h == 0:
                    eg_transposes()

                # ---- va = (x @ A_h) * EG, with EG appended as col 768 ----
                for tt in range(SD):
                    t_sl = slice(tt * P, (tt + 1) * P)
                    va0 = mmps.tile([P, 512], f32, tag="mm")
                    va1 = mmps.tile([P, 256], f32, tag="mm")
                    for dt in range(ED):
                        sx = x16[:, dt, t_sl]
                        nc.tensor.matmul(va0[:], sx, aw[:, dt, 0:512],
                                         start=(dt == 0), stop=(dt == ED - 1))
                        nc.tensor.matmul(va1[:], sx, aw[:, dt, 512:768],
                                         start=(dt == 0), stop=(dt == ED - 1))
                    sc = egt[:, tt, h:h + 1]
                    nc.scalar.mul(va_sb[:, tt, 0:512], va0[:], sc)
                    nc.scalar.mul(va_sb[:, tt, 512:768], va1[:], sc)
                    nc.vector.tensor_copy(va_sb[:, tt, 768:769], sc)

                # ---- scores / softmax / transpose / out-proj, pipelined ----
                def tail(st, ptile):
                    s_sl = slice(st * P, (st + 1) * P)
                    pTs = work.tile([P, SD, P], bf16, tag="pT")
                    for tt in range(SD):
                        t_sl = slice(tt * P, (tt + 1) * P)
                        tpp = mmps.tile([P, 512], bf16, tag="mm")
                        nc.tensor.transpose(tpp[:, 0:P], ptile[:, t_sl],
                                            ident[:])
                        nc.vector.tensor_copy(pTs[:, tt, :], tpp[:, 0:P])
                    pr = prps.tile([P, D + 1], f32, tag="pr")
                    for tt in range(SD):
                        sp = pTs[:, tt, :]
                        nc.tensor.matmul(pr[:, 0:512], sp,
                                         va_sb[:, tt, 0:512],
                                         start=(tt == 0), stop=False)
                        nc.tensor.matmul(pr[:, 512:769], sp,
                                         va_sb[:, tt, 512:769],
                                         start=(tt == 0), stop=(tt == SD - 1))
                    rc = small.tile([P, 1], f32, tag="rc")
                    nc.vector.reciprocal(rc[:], pr[:, D:D + 1])
                    if h == 0:
                        nc.scalar.mul(acc[:, st, :], pr[:, 0:D], rc[:])
                    else:
                        tmp = work.tile([P, D], f32, tag="tmp", bufs=1)
                        nc.scalar.mul(tmp[:], pr[:, 0:D], rc[:])
                        nc.vector.tensor_add(acc[:, st, :], acc[:, st, :],
                                             tmp[:])
                    if h == H - 1:
                        nc.sync.dma_start(out_t[:, st, :], acc[:, st, :])

                prev = None
                for st in range(SD):
                    s_sl = slice(st * P, (st + 1) * P)
                    sc_ps = scps.tile([P, S], f32, tag="sc")
                    for et in range(ED):
                        sf = tf32[:, et, s_sl]
                        nc.tensor.matmul(sc_ps[:, 0:512], sf,
                                         xf[:, et, 0:512],
                                         start=(et == 0), stop=False)
                        nc.tensor.matmul(sc_ps[:, 512:1024], sf,
                                         xf[:, et, 512:1024],
                                         start=(et == 0), stop=False)
                    for ep in range(ED // 2):
                        e_2 = slice(2 * ep, 2 * ep + 2)
                        s8 = t8[:, e_2, s_sl]
                        nc.tensor.matmul(sc_ps[:, 0:512], s8,
                                         x8[:, e_2, 0:512],
                                         start=False, stop=(ep == 2),
                                         perf_mode=DR)
                        nc.tensor.matmul(sc_ps[:, 512:1024], s8,
                                         x8[:, e_2, 512:1024],
                                         start=False, stop=(ep == 2),
                                         perf_mode=DR)
                    negmax = small.tile([P, 1], f32, tag="negmax")
                    nc.vector.tensor_reduce(
                        negmax[:], sc_ps[:], axis=mybir.AxisListType.X,
                        op=mybir.AluOpType.max, negate=True)
                    ptile = work.tile([P, S], bf16, tag="p")
                    nc.scalar.activation(ptile[:], sc_ps[:], AF.Exp,
                                         bias=negmax[:])
                    if prev is not None:
                        tail(*prev)
                    prev = (st, ptile)
                tail(*prev)

    nc.compile()
    return nc


def _get_nc():
    if "nc" not in _CACHE:
        _CACHE["nc"] = _build_nc()
    return _CACHE["nc"]


def _q12(a):
    """Round fp32 mantissa to 12 bits RNE (matches TRN2 fp32r operand quant)."""
    a32 = np.asarray(a, np.float32)
    c = np.float32(2 ** 12 + 1)
    s = (a32 * c).astype(np.float32)
    return (s - (s - a32).astype(np.float32)).astype(np.float32)


def _prepare(x, Wq, bq, Wk, bk, Wv, bv, Wp, bp):
    x = np.asarray(x, dtype=np.float32)
    Wq = np.asarray(Wq, dtype=np.float32)
    Wk = np.asarray(Wk, dtype=np.float32)
    Wv = np.asarray(Wv, dtype=np.float32)
    Wp = np.asarray(Wp, dtype=np.float32)
    bq = np.asarray(bq, dtype=np.float32)
    bv = np.asarray(bv, dtype=np.float32)
    bp = np.asarray(bp, dtype=np.float32)

    wp3 = Wp.reshape(H, D, D)
    M8 = 8.0 * np.matmul(Wq, np.transpose(Wk, (0, 2, 1)))
    A = np.matmul(Wv, wp3)
    c8 = 8.0 * np.einsum('hde,he->hd', Wk, bq)
    bp_eff = (bp.astype(np.float64)
              + np.einsum('hd,hde->e', bv.astype(np.float64),
                          wp3.astype(np.float64))).astype(np.float32)

    m_8 = (M8 - _q12(M8)).astype(ml_dtypes.float8_e5m2)
    a_16 = A.astype(np.float16)
    cT_f = np.ascontiguousarray(c8.T)  # [D, H] fp32

    shared = {
        "m_f": M8, "m_8": m_8, "a_w": a_16, "cT_f": cT_f,
    }
    in_maps = []
    for b in range(B):
        xT = np.ascontiguousarray(x[b].T)
        m = {"xT_f": xT, "xT_16": xT.astype(np.float16),
             "xT_8": xT.astype(ml_dtypes.float8_e5m2), **shared}
        in_maps.append(m)
    return in_maps, bp_eff


def kernel(x, Wq, bq, Wk, bk, Wv, bv, Wp, bp):
    from concourse.bass_utils import run_bass_kernel_spmd

    in_maps, bp_eff = _prepare(x, Wq, bq, Wk, bk, Wv, bv, Wp, bp)
    nc = _get_nc()
    res = run_bass_kernel_spmd(nc, in_maps, list(range(B)))
    out = np.stack([res.results[b]["out"] for b in range(B)], axis=0)
    out = out + bp_eff[None, None, :]
    return out.astype(np.float32)
